# revision 69
# baseline (speedup 1.0000x reference)
"""MultiHeadAttention (conv1x1+BN projections, masked softmax) on 8 TRN2 cores.

Sharding: pure data-parallel — batch b -> core b (B=8, n_cores=8), no
collectives. BatchNorm is folded into the projection weights host-side.
Device program per core (C=512, T=1024, H=8 heads, d_k=64):
  1. q/k/v projections: folded-weight matmuls (fp32r) + bias eviction
  2. per head: scores = k^T q (PSUM), exp(0.125*scores) on ACT,
     multiply by {0,1} mask on DVE/Pool (equivalent to -inf fill +
     post-softmax re-mask), mm2 with a ones-augmented v^T stationary
     so PSUM row 64 accumulates the softmax denominator
  3. normalize by reciprocal of the denominator (broadcast via K=1 matmul)
  4. output projection + bias, DMA out
new_cache = concat(raw key, raw value) is pure data movement -> host-side.
"""

import numpy as np

B, C, T, H = 8, 512, 1024, 8
DK = C // H          # 64
P = 128              # SBUF partitions
NC = 4               # C // P channel chunks
EPS = 1e-5

_CACHE = {}


def _get_mods():
    if "mods" not in _CACHE:
        try:
            import concourse.bass as bass  # noqa
        except ImportError:
            import sys
            sys.path.insert(0, "/opt/trn_rl_repo")
        import concourse.bass as bass
        import concourse.tile as tile
        from concourse import bacc, mybir
        from concourse import bass_utils
        from concourse.masks import make_identity
        _CACHE["mods"] = (bass, tile, bacc, mybir, bass_utils, make_identity)
    return _CACHE["mods"]


def _build_program():
    bass, tile, bacc, mybir, bass_utils, make_identity = _get_mods()
    f32 = mybir.dt.float32
    f32r = mybir.dt.float32r
    f16 = mybir.dt.float16

    def r(ap):
        return ap.bitcast(f32r)

    nc = bacc.Bacc("TRN2", target_bir_lowering=False, debug=False)

    x_q = nc.dram_tensor("x_q", [C, T], f32, kind="ExternalInput")
    x_k = nc.dram_tensor("x_k", [C, T], f32, kind="ExternalInput")
    x_v = nc.dram_tensor("x_v", [C, T], f32, kind="ExternalInput")
    m_t = nc.dram_tensor("m_t", [T, T], f16, kind="ExternalInput")
    w_pk = nc.dram_tensor("w_pk", [P, 4 * NC * C], f32, kind="ExternalInput")
    b_pk = nc.dram_tensor("b_pk", [P, 16], f32, kind="ExternalInput")
    out_d = nc.dram_tensor("out_d", [C, T], f32, kind="ExternalOutput")

    with tile.TileContext(nc) as tc:
        with nc.allow_low_precision(reason="fp32r rounding + bf16 attention weights"):
            # ---- persistent SBUF tensors ----
            _keep = []  # hold free() closures so single pools aren't GC-released

            def single(shape, name, dtype=f32):
                t, free = tc.tile(shape, dtype, name=name)
                _keep.append(free)
                return t

            w_sb = single([P, 4 * NC * C], "w_sb")       # 32KB/p
            b_sb = single([P, 16], "b_sb")
            ident = single([P, P], "ident")
            ones_sb = single([P, DK], "ones_sb")
            q_sb = single([P, NC * T], "q_sb")           # 16KB/p
            k_sb = single([P, NC * T], "k_sb")
            vt_sb = single([P, H * 8 * (DK + 1)], "vt_sb", f16)
            m_sb = single([P, 8 * T], "m_sb", f16)       # 16KB/p
            x_sb = single([P, NC * T], "x_sb")
            bias_e = single([P, 1], "bias_e")            # exp range-shift bias

            make_identity(nc, ident[:, :])
            nc.gpsimd.memset(ones_sb[:, :], 1.0)
            nc.gpsimd.memset(bias_e[:, :], -1.5)
            # fp16 ones column (denominator trick) in every 65-wide vt block
            nc.vector.tensor_copy(vt_sb[:, DK::DK + 1], ones_sb[:, :])

            nc.sync.dma_start(b_sb[:, :], b_pk[:, :])

            # ---------- phase 1: q/k/v projections (+ v transpose) ----------
            with tc.tile_pool(name="ps1", bufs=1, space=bass.MemorySpace.PSUM) as ps1:
                v_sb, v_free = tc.tile([P, NC * T], f32, name="v_sb")
                with tc.tile_pool(name="xin", bufs=2) as xin_pool:
                    proj_srcs = [(x_q, q_sb, 0), (x_k, k_sb, 1), (x_v, v_sb, 2)]
                    for dram_in, dest, pi in proj_srcs:
                        # per-proj weight DMA so the first matmul doesn't
                        # wait on the full 4MB weight transfer
                        nc.sync.dma_start(
                            r(w_sb[:, pi * 2048:(pi + 1) * 2048]),
                            r(w_pk[:, pi * 2048:(pi + 1) * 2048]),
                        )
                        xt = xin_pool.tile([P, NC * T], f32, name="xt_in", tag="xin")
                        for kk in range(NC):
                            nc.sync.dma_start(
                                r(xt[:, kk * T:(kk + 1) * T]),
                                r(dram_in[kk * P:(kk + 1) * P, :]),
                            )
                        for m in range(NC):
                            pp = ps1.tile([P, T], f32, name="pp", tag="pp", bufs=2)
                            for half in range(2):
                                for kk in range(NC):
                                    nc.tensor.matmul(
                                        pp[:, half * 512:(half + 1) * 512],
                                        r(w_sb[:, pi * 2048 + kk * 512 + m * P:
                                               pi * 2048 + kk * 512 + m * P + P]),
                                        r(xt[:, kk * T + half * 512:
                                             kk * T + (half + 1) * 512]),
                                        start=(kk == 0), stop=(kk == NC - 1),
                                    )
                            ev = dest[:, m * T:(m + 1) * T]
                            if pi < 2:
                                ev = r(ev)
                            nc.vector.tensor_scalar_add(
                                ev, pp[:, :],
                                b_sb[:, pi * 4 + m: pi * 4 + m + 1],
                            )
                    # mask + o-proj weight DMAs after input DMAs so q/k/v
                    # aren't delayed
                    for s in range(8):
                        nc.sync.dma_start(
                            m_sb[:, s * T:(s + 1) * T],
                            m_t[s * P:(s + 1) * P, :],
                        )
                    nc.sync.dma_start(
                        r(w_sb[:, 3 * 2048:4 * 2048]),
                        r(w_pk[:, 3 * 2048:4 * 2048]),
                    )
                    # v transposes: head pair (2c, 2c+1) per c-chunk
                    for c in range(NC):
                        for s in range(8):
                            tp = ps1.tile([P, P], f32, name="tp", tag="tp", bufs=2)
                            nc.tensor.transpose(
                                tp[:, :], v_sb[:, c * T + s * P: c * T + (s + 1) * P],
                                ident[:, :],
                            )
                            for j in range(2):
                                h = 2 * c + j
                                nc.vector.tensor_copy(
                                    vt_sb[:, h * 520 + s * 65: h * 520 + s * 65 + DK],
                                    tp[:, j * DK:(j + 1) * DK],
                                )
                v_free()

            # reciprocal staging (rc_tmp plain f32 -> rc_f f32r) + f32r ones
            rc_den = single([1, 2 * T], "rc_den")
            rc_tmp = single([1, 2 * T], "rc_tmp")
            rc_f = single([1, 2 * T], "rc_f")
            ones1r = single([1, DK], "ones1r")
            nc.vector.tensor_copy(r(ones1r[0:1, :]), ones_sb[0:1, :])

            # ---------- phase 2: attention ----------
            Exp = mybir.ActivationFunctionType.Exp
            with tc.tile_pool(name="ps2", bufs=1, space=bass.MemorySpace.PSUM) as ps2, \
                 tc.tile_pool(name="at", bufs=3) as at_pool, \
                 tc.tile_pool(name="xt", bufs=2) as xt_pool:
                for h in range(H):
                    po = (h % 2) * DK          # partition offset of this head
                    co = (h // 2) * T          # column offset (c-chunk) of this head
                    xa = ps2.tile([P, T], f32, name="xa", tag="x", bufs=2)
                    for s in range(8):
                        sc = ps2.tile([P, T], f32, name="sc", tag="sc", bufs=2)
                        for half in range(2):
                            nc.tensor.matmul(
                                sc[:, half * 512:(half + 1) * 512],
                                r(k_sb[po:po + DK, co + s * P: co + (s + 1) * P]),
                                r(q_sb[po:po + DK, co + half * 512: co + (half + 1) * 512]),
                            )
                        at = at_pool.tile([P, T], f16, name="at_t", tag="at")
                        nc.scalar.activation(at[:, :], sc[:, :], Exp,
                                             scale=0.125, bias=bias_e[:, 0:1])
                        nc.vector.tensor_mul(
                            at[:, :], at[:, :], m_sb[:, s * T:(s + 1) * T],
                        )
                        for half in range(2):
                            nc.tensor.matmul(
                                xa[0:DK + 1, half * 512:(half + 1) * 512],
                                vt_sb[:, h * 520 + s * 65: h * 520 + (s + 1) * 65],
                                at[:, half * 512:(half + 1) * 512],
                                start=(s == 0), stop=(s == 7),
                            )
                    # stage denominator at partition 0 (custom DVE op
                    # ignores input partition base), then fast reciprocal
                    rco = (h % 2) * T
                    nc.vector.tensor_copy(
                        rc_den[0:1, rco:rco + T], xa[DK:DK + 1, :],
                    )
                    nc.vector.reciprocal_approx_fast(
                        rc_tmp[0:1, rco:rco + T], rc_den[0:1, rco:rco + T],
                    )
                    nc.vector.tensor_copy(
                        r(rc_f[0:1, rco:rco + T]), rc_tmp[0:1, rco:rco + T],
                    )
                    xt2 = xt_pool.tile([P, T], f32, name="xt2", tag="xt")
                    nc.vector.tensor_copy(xt2[0:DK, :], xa[0:DK, :])
                    bc = ps2.tile([P, T], f32, name="bc", tag="x", bufs=2)
                    for half in range(2):
                        nc.tensor.matmul(
                            bc[0:DK, half * 512:(half + 1) * 512],
                            r(ones1r[0:1, :]),
                            r(rc_f[0:1, rco + half * 512:rco + (half + 1) * 512]),
                        )
                    nc.vector.tensor_mul(
                        r(x_sb[po:po + DK, co:co + T]),
                        xt2[0:DK, :], bc[0:DK, :],
                    )

            # ---------- phase 3: output projection ----------
            with tc.tile_pool(name="ps3", bufs=1, space=bass.MemorySpace.PSUM) as ps3, \
                 tc.tile_pool(name="outp", bufs=2) as out_pool:
                for m in range(NC):
                    pp = ps3.tile([P, T], f32, name="pp3", tag="po", bufs=2)
                    for half in range(2):
                        for kk in range(NC):
                            nc.tensor.matmul(
                                pp[:, half * 512:(half + 1) * 512],
                                r(w_sb[:, 3 * 2048 + kk * 512 + m * P:
                                       3 * 2048 + kk * 512 + m * P + P]),
                                r(x_sb[:, kk * T + half * 512:
                                       kk * T + (half + 1) * 512]),
                                start=(kk == 0), stop=(kk == NC - 1),
                            )
                    ot = out_pool.tile([P, T], f32, name="ot", tag="out")
                    nc.vector.tensor_scalar_add(
                        ot[:, :], pp[:, :], b_sb[:, 12 + m: 12 + m + 1],
                    )
                    nc.sync.dma_start(out_d[m * P:(m + 1) * P, :], ot[:, :])

            # release singles LIFO so no TilePoolBoundary leaks into BIR
            for free in reversed(_keep):
                free()

    nc.compile()
    return nc


def _get_program():
    if "nc" not in _CACHE:
        _CACHE["nc"] = _build_program()
    return _CACHE["nc"]


def _prep_in_maps(query, key, value, mask, Ws, bs, gammas, betas, means, vars_):
    f = np.float32
    inv = (gammas / np.sqrt(vars_ + EPS)).astype(f)            # [4,C]
    Wf = (Ws * inv[:, :, None]).astype(f)                      # [4,o,c]
    bf = (bs * inv + betas - means * inv).astype(f)            # [4,C]
    # w_pk[p, pi*2048 + kk*512 + m] = Wf[pi].T[kk*128+p, m]
    wT = np.transpose(Wf, (0, 2, 1))                           # [4, c_in, c_out]
    w_pack = np.ascontiguousarray(
        wT.reshape(4, NC, P, C).transpose(2, 0, 1, 3).reshape(P, 4 * NC * C)
    ).astype(f)
    b_pack = np.ascontiguousarray(
        bf.reshape(4, NC, P).transpose(2, 0, 1).reshape(P, 16)
    ).astype(f)

    q_in = np.ascontiguousarray(query.reshape(B, C, T).astype(f))
    k_in = np.ascontiguousarray(key.reshape(B, C, T).astype(f))
    v_in = np.ascontiguousarray(value.reshape(B, C, T).astype(f))
    in_maps = []
    for b in range(B):
        mT = np.ascontiguousarray(mask[b, 0].T.astype(np.float16))
        in_maps.append({
            "x_q": q_in[b], "x_k": k_in[b], "x_v": v_in[b],
            "m_t": mT, "w_pk": w_pack, "b_pk": b_pack,
        })
    return in_maps


def run(inputs, trace=False, **kw):
    _, _, _, _, bass_utils, _ = _get_mods()
    nc = _get_program()
    in_maps = _prep_in_maps(**inputs)
    res = bass_utils.run_bass_kernel_spmd(
        nc, in_maps, core_ids=list(range(B)), trace=trace, **kw,
    )
    out = np.stack([res.results[b]["out_d"] for b in range(B)], axis=0)
    out = out.reshape(B, C, 1, T).astype(np.float32)
    new_cache = np.concatenate(
        [np.asarray(inputs["key"], dtype=np.float32),
         np.asarray(inputs["value"], dtype=np.float32)], axis=1,
    )
    return (out, new_cache), res


def kernel(**inputs):
    outs, _ = run(inputs, trace=False)
    return outs



# revision 70
# speedup vs baseline: 1.1697x; 1.1697x over previous
"""MultiHeadAttention (conv1x1+BN projections, masked softmax) on 8 TRN2 cores.

Sharding: pure data-parallel — batch b -> core b (B=8, n_cores=8), no
collectives. BatchNorm is folded into the projection weights host-side.
Device program per core (C=512, T=1024, H=8 heads, d_k=64):
  1. q/k/v projections: folded-weight matmuls (fp32r) + bias eviction
  2. per head: scores = k^T q (PSUM), exp(0.125*scores) on ACT,
     multiply by {0,1} mask on DVE/Pool (equivalent to -inf fill +
     post-softmax re-mask), mm2 with a ones-augmented v^T stationary
     so PSUM row 64 accumulates the softmax denominator
  3. normalize by reciprocal of the denominator (broadcast via K=1 matmul)
  4. output projection + bias, DMA out
new_cache = concat(raw key, raw value) is pure data movement -> host-side.
"""

import numpy as np

B, C, T, H = 8, 512, 1024, 8
DK = C // H          # 64
P = 128              # SBUF partitions
NC = 4               # C // P channel chunks
EPS = 1e-5

_CACHE = {}


def _get_mods():
    if "mods" not in _CACHE:
        try:
            import concourse.bass as bass  # noqa
        except ImportError:
            import sys
            sys.path.insert(0, "/opt/trn_rl_repo")
        import concourse.bass as bass
        import concourse.tile as tile
        from concourse import bacc, mybir
        from concourse import bass_utils
        from concourse.masks import make_identity
        _CACHE["mods"] = (bass, tile, bacc, mybir, bass_utils, make_identity)
    return _CACHE["mods"]


def _build_program():
    bass, tile, bacc, mybir, bass_utils, make_identity = _get_mods()
    f32 = mybir.dt.float32
    f32r = mybir.dt.float32r
    f16 = mybir.dt.float16

    def r(ap):
        return ap.bitcast(f32r)

    nc = bacc.Bacc("TRN2", target_bir_lowering=False, debug=False)

    x_q = nc.dram_tensor("x_q", [C, T], f32, kind="ExternalInput")
    x_k = nc.dram_tensor("x_k", [C, T], f32, kind="ExternalInput")
    x_v = nc.dram_tensor("x_v", [C, T], f32, kind="ExternalInput")
    m_t = nc.dram_tensor("m_t", [T, T], f16, kind="ExternalInput")
    w_pk = nc.dram_tensor("w_pk", [P, 4 * NC * C], f32, kind="ExternalInput")
    b_pk = nc.dram_tensor("b_pk", [P, 16], f32, kind="ExternalInput")
    out_d = nc.dram_tensor("out_d", [C, T], f32, kind="ExternalOutput")

    with tile.TileContext(nc) as tc:
        with nc.allow_low_precision(reason="fp32r rounding + bf16 attention weights"):
            # ---- persistent SBUF tensors ----
            _keep = []  # hold free() closures so single pools aren't GC-released

            def single(shape, name, dtype=f32):
                t, free = tc.tile(shape, dtype, name=name)
                _keep.append(free)
                return t

            w_sb = single([P, 4 * NC * C], "w_sb")       # 32KB/p
            b_sb = single([P, 16], "b_sb")
            ident = single([P, P], "ident")
            ones_sb = single([P, DK], "ones_sb")
            q_sb = single([P, NC * T], "q_sb")           # 16KB/p
            k_sb = single([P, NC * T], "k_sb")
            vt_sb = single([P, H * 8 * (DK + 1)], "vt_sb", f16)
            m_sb = single([P, 8 * T], "m_sb", f16)       # 16KB/p
            x_sb = single([P, NC * T], "x_sb")
            bias_e = single([P, 1], "bias_e")            # exp range-shift bias

            make_identity(nc, ident[:, :])
            nc.gpsimd.memset(ones_sb[:, :], 1.0)
            nc.gpsimd.memset(bias_e[:, :], -1.5)
            # fp16 ones column (denominator trick) in every 65-wide vt block
            nc.vector.tensor_copy(vt_sb[:, DK::DK + 1], ones_sb[:, :])

            nc.sync.dma_start(b_sb[:, :], b_pk[:, :])

            # ---------- phase 1: q/k/v projections (+ v transpose) ----------
            with tc.tile_pool(name="ps1", bufs=1, space=bass.MemorySpace.PSUM) as ps1:
                v_sb, v_free = tc.tile([P, NC * T], f32, name="v_sb")
                with tc.tile_pool(name="xin", bufs=2) as xin_pool:
                    proj_srcs = [(x_q, q_sb, 0), (x_k, k_sb, 1), (x_v, v_sb, 2)]
                    for dram_in, dest, pi in proj_srcs:
                        # per-proj weight DMA so the first matmul doesn't
                        # wait on the full 4MB weight transfer
                        nc.sync.dma_start(
                            r(w_sb[:, pi * 2048:(pi + 1) * 2048]),
                            r(w_pk[:, pi * 2048:(pi + 1) * 2048]),
                        )
                        xt = xin_pool.tile([P, NC * T], f32, name="xt_in", tag="xin")
                        for kk in range(NC):
                            nc.sync.dma_start(
                                r(xt[:, kk * T:(kk + 1) * T]),
                                r(dram_in[kk * P:(kk + 1) * P, :]),
                            )
                        for m in range(NC):
                            pp = ps1.tile([P, T], f32, name="pp", tag="pp", bufs=2)
                            for half in range(2):
                                for kk in range(NC):
                                    nc.tensor.matmul(
                                        pp[:, half * 512:(half + 1) * 512],
                                        r(w_sb[:, pi * 2048 + kk * 512 + m * P:
                                               pi * 2048 + kk * 512 + m * P + P]),
                                        r(xt[:, kk * T + half * 512:
                                             kk * T + (half + 1) * 512]),
                                        start=(kk == 0), stop=(kk == NC - 1),
                                    )
                            ev = dest[:, m * T:(m + 1) * T]
                            if pi < 2:
                                ev = r(ev)
                            nc.vector.tensor_scalar_add(
                                ev, pp[:, :],
                                b_sb[:, pi * 4 + m: pi * 4 + m + 1],
                            )
                    # mask + o-proj weight DMAs after input DMAs so q/k/v
                    # aren't delayed
                    for s in range(8):
                        nc.sync.dma_start(
                            m_sb[:, s * T:(s + 1) * T],
                            m_t[s * P:(s + 1) * P, :],
                        )
                    nc.sync.dma_start(
                        r(w_sb[:, 3 * 2048:4 * 2048]),
                        r(w_pk[:, 3 * 2048:4 * 2048]),
                    )
                    # v transposes: head pair (2c, 2c+1) per c-chunk
                    for c in range(NC):
                        for s in range(8):
                            tp = ps1.tile([P, P], f32, name="tp", tag="tp", bufs=2)
                            nc.tensor.transpose(
                                tp[:, :], v_sb[:, c * T + s * P: c * T + (s + 1) * P],
                                ident[:, :],
                            )
                            for j in range(2):
                                h = 2 * c + j
                                nc.vector.tensor_copy(
                                    vt_sb[:, h * 520 + s * 65: h * 520 + s * 65 + DK],
                                    tp[:, j * DK:(j + 1) * DK],
                                )
                v_free()

            # reciprocal staging (rc_tmp plain f32 -> rc_f f32r) + f32r ones
            rc_den = single([1, 2 * T], "rc_den")
            rc_tmp = single([1, 2 * T], "rc_tmp")
            rc_f = single([1, 2 * T], "rc_f")
            ones1r = single([1, DK], "ones1r")
            nc.vector.tensor_copy(r(ones1r[0:1, :]), ones_sb[0:1, :])

            # ---------- phase 2: attention ----------
            Exp = mybir.ActivationFunctionType.Exp
            with tc.tile_pool(name="ps2", bufs=1, space=bass.MemorySpace.PSUM) as ps2, \
                 tc.tile_pool(name="at", bufs=3) as at_pool, \
                 tc.tile_pool(name="xt", bufs=2) as xt_pool:
                for h in range(H):
                    po = (h % 2) * DK          # partition offset of this head
                    co = (h // 2) * T          # column offset (c-chunk) of this head
                    xa = ps2.tile([P, T], f32, name="xa", tag="x", bufs=2)
                    for s in range(8):
                        sc = ps2.tile([P, T], f32, name="sc", tag="sc", bufs=2)
                        for half in range(2):
                            nc.tensor.matmul(
                                sc[:, half * 512:(half + 1) * 512],
                                r(k_sb[po:po + DK, co + s * P: co + (s + 1) * P]),
                                r(q_sb[po:po + DK, co + half * 512: co + (half + 1) * 512]),
                            )
                        at = at_pool.tile([P, T], f16, name="at_t", tag="at")
                        nc.scalar.activation(at[:, :], sc[:, :], Exp,
                                             scale=0.125, bias=bias_e[:, 0:1])
                        nc.vector.tensor_mul(
                            at[:, :], at[:, :], m_sb[:, s * T:(s + 1) * T],
                        )
                        for half in range(2):
                            nc.tensor.matmul(
                                xa[0:DK + 1, half * 512:(half + 1) * 512],
                                vt_sb[:, h * 520 + s * 65: h * 520 + (s + 1) * 65],
                                at[:, half * 512:(half + 1) * 512],
                                start=(s == 0), stop=(s == 7),
                            )
                    # wide PSUM evict, then stage denominator at partition 0
                    # via cheap SBUF->SBUF copy (custom DVE recip op ignores
                    # input partition base; 1p PSUM reads are ~5x slower)
                    xt2 = xt_pool.tile([P, T], f32, name="xt2", tag="xt")
                    nc.vector.tensor_copy(xt2[0:DK + 1, :], xa[0:DK + 1, :])
                    rco = (h % 2) * T
                    nc.vector.tensor_copy(
                        rc_den[0:1, rco:rco + T], xt2[DK:DK + 1, :],
                    )
                    nc.vector.reciprocal_approx_fast(
                        rc_tmp[0:1, rco:rco + T], rc_den[0:1, rco:rco + T],
                    )
                    nc.vector.tensor_copy(
                        r(rc_f[0:1, rco:rco + T]), rc_tmp[0:1, rco:rco + T],
                    )
                    bc = ps2.tile([P, T], f32, name="bc", tag="x", bufs=2)
                    for half in range(2):
                        nc.tensor.matmul(
                            bc[0:DK, half * 512:(half + 1) * 512],
                            r(ones1r[0:1, :]),
                            r(rc_f[0:1, rco + half * 512:rco + (half + 1) * 512]),
                        )
                    nc.vector.tensor_mul(
                        r(x_sb[po:po + DK, co:co + T]),
                        xt2[0:DK, :], bc[0:DK, :],
                    )

            # ---------- phase 3: output projection ----------
            with tc.tile_pool(name="ps3", bufs=1, space=bass.MemorySpace.PSUM) as ps3, \
                 tc.tile_pool(name="outp", bufs=2) as out_pool:
                for m in range(NC):
                    pp = ps3.tile([P, T], f32, name="pp3", tag="po", bufs=2)
                    for half in range(2):
                        for kk in range(NC):
                            nc.tensor.matmul(
                                pp[:, half * 512:(half + 1) * 512],
                                r(w_sb[:, 3 * 2048 + kk * 512 + m * P:
                                       3 * 2048 + kk * 512 + m * P + P]),
                                r(x_sb[:, kk * T + half * 512:
                                       kk * T + (half + 1) * 512]),
                                start=(kk == 0), stop=(kk == NC - 1),
                            )
                    ot = out_pool.tile([P, T], f32, name="ot", tag="out")
                    nc.vector.tensor_scalar_add(
                        ot[:, :], pp[:, :], b_sb[:, 12 + m: 12 + m + 1],
                    )
                    nc.sync.dma_start(out_d[m * P:(m + 1) * P, :], ot[:, :])

            # release singles LIFO so no TilePoolBoundary leaks into BIR
            for free in reversed(_keep):
                free()

    nc.compile()
    return nc


def _get_program():
    if "nc" not in _CACHE:
        _CACHE["nc"] = _build_program()
    return _CACHE["nc"]


def _prep_in_maps(query, key, value, mask, Ws, bs, gammas, betas, means, vars_):
    f = np.float32
    inv = (gammas / np.sqrt(vars_ + EPS)).astype(f)            # [4,C]
    Wf = (Ws * inv[:, :, None]).astype(f)                      # [4,o,c]
    bf = (bs * inv + betas - means * inv).astype(f)            # [4,C]
    # w_pk[p, pi*2048 + kk*512 + m] = Wf[pi].T[kk*128+p, m]
    wT = np.transpose(Wf, (0, 2, 1))                           # [4, c_in, c_out]
    w_pack = np.ascontiguousarray(
        wT.reshape(4, NC, P, C).transpose(2, 0, 1, 3).reshape(P, 4 * NC * C)
    ).astype(f)
    b_pack = np.ascontiguousarray(
        bf.reshape(4, NC, P).transpose(2, 0, 1).reshape(P, 16)
    ).astype(f)

    q_in = np.ascontiguousarray(query.reshape(B, C, T).astype(f))
    k_in = np.ascontiguousarray(key.reshape(B, C, T).astype(f))
    v_in = np.ascontiguousarray(value.reshape(B, C, T).astype(f))
    in_maps = []
    for b in range(B):
        mT = np.ascontiguousarray(mask[b, 0].T.astype(np.float16))
        in_maps.append({
            "x_q": q_in[b], "x_k": k_in[b], "x_v": v_in[b],
            "m_t": mT, "w_pk": w_pack, "b_pk": b_pack,
        })
    return in_maps


def run(inputs, trace=False, **kw):
    _, _, _, _, bass_utils, _ = _get_mods()
    nc = _get_program()
    in_maps = _prep_in_maps(**inputs)
    res = bass_utils.run_bass_kernel_spmd(
        nc, in_maps, core_ids=list(range(B)), trace=trace, **kw,
    )
    out = np.stack([res.results[b]["out_d"] for b in range(B)], axis=0)
    out = out.reshape(B, C, 1, T).astype(np.float32)
    new_cache = np.concatenate(
        [np.asarray(inputs["key"], dtype=np.float32),
         np.asarray(inputs["value"], dtype=np.float32)], axis=1,
    )
    return (out, new_cache), res


def kernel(**inputs):
    outs, _ = run(inputs, trace=False)
    return outs



# revision 76
# speedup vs baseline: 1.1823x; 1.0108x over previous
"""MultiHeadAttention (conv1x1+BN projections, masked softmax) on 8 TRN2 cores.

Sharding: pure data-parallel — batch b -> core b (B=8, n_cores=8), no
collectives. BatchNorm is folded into the projection weights host-side.
Device program per core (C=512, T=1024, H=8 heads, d_k=64):
  1. q/k/v projections: folded-weight matmuls (fp32r) + bias eviction
  2. per head: scores = k^T q (PSUM), exp(0.125*scores) on ACT,
     multiply by {0,1} mask on DVE/Pool (equivalent to -inf fill +
     post-softmax re-mask), mm2 with a ones-augmented v^T stationary
     so PSUM row 64 accumulates the softmax denominator
  3. normalize by reciprocal of the denominator (broadcast via K=1 matmul)
  4. output projection + bias, DMA out
new_cache = concat(raw key, raw value) is pure data movement -> host-side.
"""

import numpy as np

B, C, T, H = 8, 512, 1024, 8
DK = C // H          # 64
P = 128              # SBUF partitions
NC = 4               # C // P channel chunks
EPS = 1e-5

_CACHE = {}


def _get_mods():
    if "mods" not in _CACHE:
        try:
            import concourse.bass as bass  # noqa
        except ImportError:
            import sys
            sys.path.insert(0, "/opt/trn_rl_repo")
        import concourse.bass as bass
        import concourse.tile as tile
        from concourse import bacc, mybir
        from concourse import bass_utils
        from concourse.masks import make_identity
        _CACHE["mods"] = (bass, tile, bacc, mybir, bass_utils, make_identity)
    return _CACHE["mods"]


def _build_program():
    bass, tile, bacc, mybir, bass_utils, make_identity = _get_mods()
    f32 = mybir.dt.float32
    f32r = mybir.dt.float32r
    f16 = mybir.dt.float16

    def r(ap):
        return ap.bitcast(f32r)

    nc = bacc.Bacc("TRN2", target_bir_lowering=False, debug=False)

    x_q = nc.dram_tensor("x_q", [C, T], f32, kind="ExternalInput")
    x_k = nc.dram_tensor("x_k", [C, T], f32, kind="ExternalInput")
    x_v = nc.dram_tensor("x_v", [C, T], f32, kind="ExternalInput")
    m_t = nc.dram_tensor("m_t", [T, T], f16, kind="ExternalInput")
    w_pk = nc.dram_tensor("w_pk", [P, 4 * NC * C], f32, kind="ExternalInput")
    b_pk = nc.dram_tensor("b_pk", [P, 16], f32, kind="ExternalInput")
    out_d = nc.dram_tensor("out_d", [C, T], f32, kind="ExternalOutput")

    with tile.TileContext(nc) as tc:
        with nc.allow_low_precision(reason="fp32r rounding + bf16 attention weights"):
            # ---- persistent SBUF tensors ----
            _keep = []  # hold free() closures so single pools aren't GC-released

            def single(shape, name, dtype=f32):
                t, free = tc.tile(shape, dtype, name=name)
                _keep.append(free)
                return t

            w_sb = single([P, 4 * NC * C], "w_sb")       # 32KB/p
            b_sb = single([P, 16], "b_sb")
            ident = single([P, P], "ident")
            ones_sb = single([P, DK], "ones_sb")
            q_sb = single([P, NC * T], "q_sb")           # 16KB/p
            k_sb = single([P, NC * T], "k_sb")
            vt_sb = single([P, H * 8 * (DK + 1)], "vt_sb", f16)
            m_sb = single([P, 8 * T], "m_sb", f16)       # 16KB/p
            x_sb = single([P, NC * T], "x_sb")
            bias_e = single([P, 1], "bias_e")            # exp range-shift bias

            make_identity(nc, ident[:, :])
            nc.gpsimd.memset(ones_sb[:, :], 1.0)
            nc.gpsimd.memset(bias_e[:, :], -1.5)
            # fp16 ones column (denominator trick) in every 65-wide vt block
            nc.vector.tensor_copy(vt_sb[:, DK::DK + 1], ones_sb[:, :])

            nc.sync.dma_start(b_sb[:, :], b_pk[:, :])

            # ---------- phase 1: q/k/v projections (+ v transpose) ----------
            with tc.tile_pool(name="ps1", bufs=1, space=bass.MemorySpace.PSUM) as ps1:
                v_sb, v_free = tc.tile([P, NC * T], f32, name="v_sb")
                with tc.tile_pool(name="xin", bufs=2) as xin_pool:
                    proj_srcs = [(x_q, q_sb, 0), (x_k, k_sb, 1), (x_v, v_sb, 2)]
                    for dram_in, dest, pi in proj_srcs:
                        # per-proj weight DMA so the first matmul doesn't
                        # wait on the full 4MB weight transfer
                        nc.sync.dma_start(
                            r(w_sb[:, pi * 2048:(pi + 1) * 2048]),
                            r(w_pk[:, pi * 2048:(pi + 1) * 2048]),
                        )
                        xt = xin_pool.tile([P, NC * T], f32, name="xt_in", tag="xin")
                        for kk in range(NC):
                            nc.sync.dma_start(
                                r(xt[:, kk * T:(kk + 1) * T]),
                                r(dram_in[kk * P:(kk + 1) * P, :]),
                            )
                        for m in range(NC):
                            pp = ps1.tile([P, T], f32, name="pp", tag="pp", bufs=2)
                            for half in range(2):
                                for kk in range(NC):
                                    nc.tensor.matmul(
                                        pp[:, half * 512:(half + 1) * 512],
                                        r(w_sb[:, pi * 2048 + kk * 512 + m * P:
                                               pi * 2048 + kk * 512 + m * P + P]),
                                        r(xt[:, kk * T + half * 512:
                                             kk * T + (half + 1) * 512]),
                                        start=(kk == 0), stop=(kk == NC - 1),
                                    )
                            ev = dest[:, m * T:(m + 1) * T]
                            if pi < 2:
                                ev = r(ev)
                            nc.vector.tensor_scalar_add(
                                ev, pp[:, :],
                                b_sb[:, pi * 4 + m: pi * 4 + m + 1],
                            )
                    # mask + o-proj weight DMAs after input DMAs so q/k/v
                    # aren't delayed
                    for s in range(8):
                        nc.sync.dma_start(
                            m_sb[:, s * T:(s + 1) * T],
                            m_t[s * P:(s + 1) * P, :],
                        )
                    nc.sync.dma_start(
                        r(w_sb[:, 3 * 2048:4 * 2048]),
                        r(w_pk[:, 3 * 2048:4 * 2048]),
                    )
                    # v transposes: head pair (2c, 2c+1) per c-chunk
                    for c in range(NC):
                        for s in range(8):
                            tp = ps1.tile([P, P], f32, name="tp", tag="tp", bufs=2)
                            nc.tensor.transpose(
                                tp[:, :], v_sb[:, c * T + s * P: c * T + (s + 1) * P],
                                ident[:, :],
                            )
                            for j in range(2):
                                h = 2 * c + j
                                nc.vector.tensor_copy(
                                    vt_sb[:, h * 520 + s * 65: h * 520 + s * 65 + DK],
                                    tp[:, j * DK:(j + 1) * DK],
                                )
                v_free()

            # reciprocal staging (rc_tmp plain f32 -> rc_f f32r) + f32r ones
            rc_den = single([1, 2 * T], "rc_den")
            rc_tmp = single([1, 2 * T], "rc_tmp")
            rc_f = single([1, 2 * T], "rc_f")
            ones1r = single([1, DK], "ones1r")
            nc.vector.tensor_copy(r(ones1r[0:1, :]), ones_sb[0:1, :])

            # ---------- phase 2: attention ----------
            Exp = mybir.ActivationFunctionType.Exp
            with tc.tile_pool(name="ps2", bufs=1, space=bass.MemorySpace.PSUM) as ps2, \
                 tc.tile_pool(name="at", bufs=3) as at_pool, \
                 tc.tile_pool(name="xt", bufs=2) as xt_pool:
                def make_chain(xa, xt2, rco):
                    # wide PSUM evict, stage denominator at partition 0 via
                    # cheap SBUF->SBUF copy (custom DVE recip op ignores
                    # input partition base; 1p PSUM reads are ~5x slower)
                    def chain():
                        nc.vector.tensor_copy(xt2[0:DK + 1, :], xa[0:DK + 1, :])
                        nc.vector.tensor_copy(
                            rc_den[0:1, rco:rco + T], xt2[DK:DK + 1, :],
                        )
                        nc.vector.reciprocal_approx_fast(
                            rc_tmp[0:1, rco:rco + T], rc_den[0:1, rco:rco + T],
                        )
                        nc.vector.tensor_copy(
                            r(rc_f[0:1, rco:rco + T]), rc_tmp[0:1, rco:rco + T],
                        )
                    return chain

                def make_bcmul(xt2, po, co, rco):
                    def bcmul():
                        bc = ps2.tile([P, T], f32, name="bc", tag="sc", bufs=2)
                        for half in range(2):
                            nc.tensor.matmul(
                                bc[0:DK, half * 512:(half + 1) * 512],
                                r(ones1r[0:1, :]),
                                r(rc_f[0:1, rco + half * 512:rco + (half + 1) * 512]),
                            )
                        nc.vector.tensor_mul(
                            r(x_sb[po:po + DK, co:co + T]),
                            xt2[0:DK, :], bc[0:DK, :],
                        )
                    return bcmul

                # previous head's finalize is deferred into the next head's
                # s-loop so the DVE chain doesn't delay the next mask mul
                chain_pend = bcmul_pend = None
                for h in range(H):
                    po = (h % 2) * DK          # partition offset of this head
                    co = (h // 2) * T          # column offset (c-chunk) of this head
                    xa = ps2.tile([P, T], f32, name="xa", tag="x", bufs=2)
                    for s in range(8):
                        sc = ps2.tile([P, T], f32, name="sc", tag="sc", bufs=2)
                        for half in range(2):
                            nc.tensor.matmul(
                                sc[:, half * 512:(half + 1) * 512],
                                r(k_sb[po:po + DK, co + s * P: co + (s + 1) * P]),
                                r(q_sb[po:po + DK, co + half * 512: co + (half + 1) * 512]),
                            )
                        at = at_pool.tile([P, T], f16, name="at_t", tag="at")
                        nc.scalar.activation(at[:, :], sc[:, :], Exp,
                                             scale=0.125, bias=bias_e[:, 0:1])
                        nc.vector.tensor_mul(
                            at[:, :], at[:, :], m_sb[:, s * T:(s + 1) * T],
                        )
                        if s == 0 and chain_pend is not None:
                            chain_pend()
                            chain_pend = None
                        if s == 3 and bcmul_pend is not None:
                            bcmul_pend()
                            bcmul_pend = None
                        for half in range(2):
                            nc.tensor.matmul(
                                xa[0:DK + 1, half * 512:(half + 1) * 512],
                                vt_sb[:, h * 520 + s * 65: h * 520 + (s + 1) * 65],
                                at[:, half * 512:(half + 1) * 512],
                                start=(s == 0), stop=(s == 7),
                            )
                    xt2 = xt_pool.tile([P, T], f32, name="xt2", tag="xt")
                    rco = (h % 2) * T
                    chain_pend = make_chain(xa, xt2, rco)
                    bcmul_pend = make_bcmul(xt2, po, co, rco)
                chain_pend()
                bcmul_pend()

            # ---------- phase 3: output projection ----------
            with tc.tile_pool(name="ps3", bufs=1, space=bass.MemorySpace.PSUM) as ps3, \
                 tc.tile_pool(name="outp", bufs=2) as out_pool:
                for m in range(NC):
                    pp = ps3.tile([P, T], f32, name="pp3", tag="po", bufs=2)
                    for half in range(2):
                        for kk in range(NC):
                            nc.tensor.matmul(
                                pp[:, half * 512:(half + 1) * 512],
                                r(w_sb[:, 3 * 2048 + kk * 512 + m * P:
                                       3 * 2048 + kk * 512 + m * P + P]),
                                r(x_sb[:, kk * T + half * 512:
                                       kk * T + (half + 1) * 512]),
                                start=(kk == 0), stop=(kk == NC - 1),
                            )
                    ot = out_pool.tile([P, T], f32, name="ot", tag="out")
                    nc.vector.tensor_scalar_add(
                        ot[:, :], pp[:, :], b_sb[:, 12 + m: 12 + m + 1],
                    )
                    nc.sync.dma_start(out_d[m * P:(m + 1) * P, :], ot[:, :])

            # release singles LIFO so no TilePoolBoundary leaks into BIR
            for free in reversed(_keep):
                free()

    nc.compile()
    return nc


def _get_program():
    if "nc" not in _CACHE:
        _CACHE["nc"] = _build_program()
    return _CACHE["nc"]


def _prep_in_maps(query, key, value, mask, Ws, bs, gammas, betas, means, vars_):
    f = np.float32
    inv = (gammas / np.sqrt(vars_ + EPS)).astype(f)            # [4,C]
    Wf = (Ws * inv[:, :, None]).astype(f)                      # [4,o,c]
    bf = (bs * inv + betas - means * inv).astype(f)            # [4,C]
    # w_pk[p, pi*2048 + kk*512 + m] = Wf[pi].T[kk*128+p, m]
    wT = np.transpose(Wf, (0, 2, 1))                           # [4, c_in, c_out]
    w_pack = np.ascontiguousarray(
        wT.reshape(4, NC, P, C).transpose(2, 0, 1, 3).reshape(P, 4 * NC * C)
    ).astype(f)
    b_pack = np.ascontiguousarray(
        bf.reshape(4, NC, P).transpose(2, 0, 1).reshape(P, 16)
    ).astype(f)

    q_in = np.ascontiguousarray(query.reshape(B, C, T).astype(f))
    k_in = np.ascontiguousarray(key.reshape(B, C, T).astype(f))
    v_in = np.ascontiguousarray(value.reshape(B, C, T).astype(f))
    in_maps = []
    for b in range(B):
        mT = np.ascontiguousarray(mask[b, 0].T.astype(np.float16))
        in_maps.append({
            "x_q": q_in[b], "x_k": k_in[b], "x_v": v_in[b],
            "m_t": mT, "w_pk": w_pack, "b_pk": b_pack,
        })
    return in_maps


def run(inputs, trace=False, **kw):
    _, _, _, _, bass_utils, _ = _get_mods()
    nc = _get_program()
    in_maps = _prep_in_maps(**inputs)
    res = bass_utils.run_bass_kernel_spmd(
        nc, in_maps, core_ids=list(range(B)), trace=trace, **kw,
    )
    out = np.stack([res.results[b]["out_d"] for b in range(B)], axis=0)
    out = out.reshape(B, C, 1, T).astype(np.float32)
    new_cache = np.concatenate(
        [np.asarray(inputs["key"], dtype=np.float32),
         np.asarray(inputs["value"], dtype=np.float32)], axis=1,
    )
    return (out, new_cache), res


def kernel(**inputs):
    outs, _ = run(inputs, trace=False)
    return outs



# revision 78
# speedup vs baseline: 1.1866x; 1.0036x over previous
"""MultiHeadAttention (conv1x1+BN projections, masked softmax) on 8 TRN2 cores.

Sharding: pure data-parallel — batch b -> core b (B=8, n_cores=8), no
collectives. BatchNorm is folded into the projection weights host-side.
Device program per core (C=512, T=1024, H=8 heads, d_k=64):
  1. q/k/v projections: folded-weight matmuls (fp32r) + bias eviction
  2. per head: scores = k^T q (PSUM), exp(0.125*scores) on ACT,
     multiply by {0,1} mask on DVE/Pool (equivalent to -inf fill +
     post-softmax re-mask), mm2 with a ones-augmented v^T stationary
     so PSUM row 64 accumulates the softmax denominator
  3. normalize by reciprocal of the denominator (broadcast via K=1 matmul)
  4. output projection + bias, DMA out
new_cache = concat(raw key, raw value) is pure data movement -> host-side.
"""

import numpy as np

B, C, T, H = 8, 512, 1024, 8
DK = C // H          # 64
P = 128              # SBUF partitions
NC = 4               # C // P channel chunks
EPS = 1e-5

_CACHE = {}


def _get_mods():
    if "mods" not in _CACHE:
        try:
            import concourse.bass as bass  # noqa
        except ImportError:
            import sys
            sys.path.insert(0, "/opt/trn_rl_repo")
        import concourse.bass as bass
        import concourse.tile as tile
        from concourse import bacc, mybir
        from concourse import bass_utils
        from concourse.masks import make_identity
        _CACHE["mods"] = (bass, tile, bacc, mybir, bass_utils, make_identity)
    return _CACHE["mods"]


def _build_program():
    bass, tile, bacc, mybir, bass_utils, make_identity = _get_mods()
    f32 = mybir.dt.float32
    f32r = mybir.dt.float32r
    f16 = mybir.dt.float16

    def r(ap):
        return ap.bitcast(f32r)

    nc = bacc.Bacc("TRN2", target_bir_lowering=False, debug=False)

    x_q = nc.dram_tensor("x_q", [C, T], f32, kind="ExternalInput")
    x_k = nc.dram_tensor("x_k", [C, T], f32, kind="ExternalInput")
    x_v = nc.dram_tensor("x_v", [C, T], f32, kind="ExternalInput")
    m_t = nc.dram_tensor("m_t", [T, T], f16, kind="ExternalInput")
    w_pk = nc.dram_tensor("w_pk", [P, 4 * NC * C], f32, kind="ExternalInput")
    b_pk = nc.dram_tensor("b_pk", [P, 16], f32, kind="ExternalInput")
    out_d = nc.dram_tensor("out_d", [C, T], f32, kind="ExternalOutput")

    with tile.TileContext(nc) as tc:
        with nc.allow_low_precision(reason="fp32r rounding + bf16 attention weights"):
            # ---- persistent SBUF tensors ----
            _keep = []  # hold free() closures so single pools aren't GC-released

            def single(shape, name, dtype=f32):
                t, free = tc.tile(shape, dtype, name=name)
                _keep.append(free)
                return t

            w_sb = single([P, 4 * NC * C], "w_sb")       # 32KB/p
            b_sb = single([P, 16], "b_sb")
            ident = single([P, P], "ident")
            ones_sb = single([P, DK], "ones_sb")
            q_sb = single([P, NC * T], "q_sb")           # 16KB/p
            k_sb = single([P, NC * T], "k_sb")
            vt_sb = single([P, H * 8 * (DK + 1)], "vt_sb", f16)
            m_sb = single([P, 8 * T], "m_sb", f16)       # 16KB/p
            x_sb = single([P, NC * T], "x_sb")
            bias_e = single([P, 1], "bias_e")            # exp range-shift bias

            make_identity(nc, ident[:, :])
            nc.gpsimd.memset(ones_sb[:, :], 1.0)
            nc.gpsimd.memset(bias_e[:, :], -1.5)
            # fp16 ones column (denominator trick) in every 65-wide vt block
            nc.vector.tensor_copy(vt_sb[:, DK::DK + 1], ones_sb[:, :])

            nc.sync.dma_start(b_sb[:, :], b_pk[:, :])

            # ---------- phase 1: q/k/v projections (+ v transpose) ----------
            with tc.tile_pool(name="ps1", bufs=1, space=bass.MemorySpace.PSUM) as ps1:
                v_sb, v_free = tc.tile([P, NC * T], f32, name="v_sb")
                with tc.tile_pool(name="xin", bufs=2) as xin_pool:
                    proj_srcs = [(x_q, q_sb, 0), (x_k, k_sb, 1), (x_v, v_sb, 2)]
                    for dram_in, dest, pi in proj_srcs:
                        # per-kk interleaved weight/input DMAs so the first
                        # matmul only waits on the first 0.75MB, not 3MB
                        xt = xin_pool.tile([P, NC * T], f32, name="xt_in", tag="xin")
                        for kk in range(NC):
                            nc.sync.dma_start(
                                r(w_sb[:, pi * 2048 + kk * 512:
                                       pi * 2048 + (kk + 1) * 512]),
                                r(w_pk[:, pi * 2048 + kk * 512:
                                       pi * 2048 + (kk + 1) * 512]),
                            )
                            nc.sync.dma_start(
                                r(xt[:, kk * T:(kk + 1) * T]),
                                r(dram_in[kk * P:(kk + 1) * P, :]),
                            )
                        for m in range(NC):
                            pp = ps1.tile([P, T], f32, name="pp", tag="pp", bufs=2)
                            for half in range(2):
                                for kk in range(NC):
                                    nc.tensor.matmul(
                                        pp[:, half * 512:(half + 1) * 512],
                                        r(w_sb[:, pi * 2048 + kk * 512 + m * P:
                                               pi * 2048 + kk * 512 + m * P + P]),
                                        r(xt[:, kk * T + half * 512:
                                             kk * T + (half + 1) * 512]),
                                        start=(kk == 0), stop=(kk == NC - 1),
                                    )
                            ev = dest[:, m * T:(m + 1) * T]
                            if pi < 2:
                                ev = r(ev)
                            nc.vector.tensor_scalar_add(
                                ev, pp[:, :],
                                b_sb[:, pi * 4 + m: pi * 4 + m + 1],
                            )
                    # mask + o-proj weight DMAs after input DMAs so q/k/v
                    # aren't delayed
                    for s in range(8):
                        nc.sync.dma_start(
                            m_sb[:, s * T:(s + 1) * T],
                            m_t[s * P:(s + 1) * P, :],
                        )
                    nc.sync.dma_start(
                        r(w_sb[:, 3 * 2048:4 * 2048]),
                        r(w_pk[:, 3 * 2048:4 * 2048]),
                    )
                    # v transposes: head pair (2c, 2c+1) per c-chunk
                    for c in range(NC):
                        for s in range(8):
                            tp = ps1.tile([P, P], f32, name="tp", tag="tp", bufs=2)
                            nc.tensor.transpose(
                                tp[:, :], v_sb[:, c * T + s * P: c * T + (s + 1) * P],
                                ident[:, :],
                            )
                            for j in range(2):
                                h = 2 * c + j
                                nc.vector.tensor_copy(
                                    vt_sb[:, h * 520 + s * 65: h * 520 + s * 65 + DK],
                                    tp[:, j * DK:(j + 1) * DK],
                                )
                v_free()

            # reciprocal staging (rc_tmp plain f32 -> rc_f f32r) + f32r ones
            rc_den = single([1, 2 * T], "rc_den")
            rc_tmp = single([1, 2 * T], "rc_tmp")
            rc_f = single([1, 2 * T], "rc_f")
            ones1r = single([1, DK], "ones1r")
            nc.vector.tensor_copy(r(ones1r[0:1, :]), ones_sb[0:1, :])

            # ---------- phase 2: attention ----------
            Exp = mybir.ActivationFunctionType.Exp
            with tc.tile_pool(name="ps2", bufs=1, space=bass.MemorySpace.PSUM) as ps2, \
                 tc.tile_pool(name="at", bufs=3) as at_pool, \
                 tc.tile_pool(name="xt", bufs=2) as xt_pool:
                def make_chain(xa, xt2, rco):
                    # wide PSUM evict, stage denominator at partition 0 via
                    # cheap SBUF->SBUF copy (custom DVE recip op ignores
                    # input partition base; 1p PSUM reads are ~5x slower)
                    def chain():
                        nc.vector.tensor_copy(xt2[0:DK + 1, :], xa[0:DK + 1, :])
                        nc.vector.tensor_copy(
                            rc_den[0:1, rco:rco + T], xt2[DK:DK + 1, :],
                        )
                        nc.vector.reciprocal_approx_fast(
                            rc_tmp[0:1, rco:rco + T], rc_den[0:1, rco:rco + T],
                        )
                        nc.vector.tensor_copy(
                            r(rc_f[0:1, rco:rco + T]), rc_tmp[0:1, rco:rco + T],
                        )
                    return chain

                def make_bcmul(xt2, po, co, rco):
                    def bcmul():
                        bc = ps2.tile([P, T], f32, name="bc", tag="sc", bufs=2)
                        for half in range(2):
                            nc.tensor.matmul(
                                bc[0:DK, half * 512:(half + 1) * 512],
                                r(ones1r[0:1, :]),
                                r(rc_f[0:1, rco + half * 512:rco + (half + 1) * 512]),
                            )
                        nc.vector.tensor_mul(
                            r(x_sb[po:po + DK, co:co + T]),
                            xt2[0:DK, :], bc[0:DK, :],
                        )
                    return bcmul

                # previous head's finalize is deferred into the next head's
                # s-loop so the DVE chain doesn't delay the next mask mul
                chain_pend = bcmul_pend = None
                for h in range(H):
                    po = (h % 2) * DK          # partition offset of this head
                    co = (h // 2) * T          # column offset (c-chunk) of this head
                    xa = ps2.tile([P, T], f32, name="xa", tag="x", bufs=2)
                    for s in range(8):
                        sc = ps2.tile([P, T], f32, name="sc", tag="sc", bufs=2)
                        for half in range(2):
                            nc.tensor.matmul(
                                sc[:, half * 512:(half + 1) * 512],
                                r(k_sb[po:po + DK, co + s * P: co + (s + 1) * P]),
                                r(q_sb[po:po + DK, co + half * 512: co + (half + 1) * 512]),
                            )
                        at = at_pool.tile([P, T], f16, name="at_t", tag="at")
                        nc.scalar.activation(at[:, :], sc[:, :], Exp,
                                             scale=0.125, bias=bias_e[:, 0:1])
                        nc.vector.tensor_mul(
                            at[:, :], at[:, :], m_sb[:, s * T:(s + 1) * T],
                        )
                        if s == 0 and chain_pend is not None:
                            chain_pend()
                            chain_pend = None
                        if s == 3 and bcmul_pend is not None:
                            bcmul_pend()
                            bcmul_pend = None
                        for half in range(2):
                            nc.tensor.matmul(
                                xa[0:DK + 1, half * 512:(half + 1) * 512],
                                vt_sb[:, h * 520 + s * 65: h * 520 + (s + 1) * 65],
                                at[:, half * 512:(half + 1) * 512],
                                start=(s == 0), stop=(s == 7),
                            )
                    xt2 = xt_pool.tile([P, T], f32, name="xt2", tag="xt")
                    rco = (h % 2) * T
                    chain_pend = make_chain(xa, xt2, rco)
                    bcmul_pend = make_bcmul(xt2, po, co, rco)
                chain_pend()
                bcmul_pend()

            # ---------- phase 3: output projection ----------
            with tc.tile_pool(name="ps3", bufs=1, space=bass.MemorySpace.PSUM) as ps3, \
                 tc.tile_pool(name="outp", bufs=3) as out_pool:
                for m in range(NC):
                    pp = ps3.tile([P, T], f32, name="pp3", tag="po", bufs=3)
                    for half in range(2):
                        for kk in range(NC):
                            nc.tensor.matmul(
                                pp[:, half * 512:(half + 1) * 512],
                                r(w_sb[:, 3 * 2048 + kk * 512 + m * P:
                                       3 * 2048 + kk * 512 + m * P + P]),
                                r(x_sb[:, kk * T + half * 512:
                                       kk * T + (half + 1) * 512]),
                                start=(kk == 0), stop=(kk == NC - 1),
                            )
                    ot = out_pool.tile([P, T], f32, name="ot", tag="out")
                    nc.vector.tensor_scalar_add(
                        ot[:, :], pp[:, :], b_sb[:, 12 + m: 12 + m + 1],
                    )
                    nc.sync.dma_start(out_d[m * P:(m + 1) * P, :], ot[:, :])

            # release singles LIFO so no TilePoolBoundary leaks into BIR
            for free in reversed(_keep):
                free()

    nc.compile()
    return nc


def _get_program():
    if "nc" not in _CACHE:
        _CACHE["nc"] = _build_program()
    return _CACHE["nc"]


def _prep_in_maps(query, key, value, mask, Ws, bs, gammas, betas, means, vars_):
    f = np.float32
    inv = (gammas / np.sqrt(vars_ + EPS)).astype(f)            # [4,C]
    Wf = (Ws * inv[:, :, None]).astype(f)                      # [4,o,c]
    bf = (bs * inv + betas - means * inv).astype(f)            # [4,C]
    # w_pk[p, pi*2048 + kk*512 + m] = Wf[pi].T[kk*128+p, m]
    wT = np.transpose(Wf, (0, 2, 1))                           # [4, c_in, c_out]
    w_pack = np.ascontiguousarray(
        wT.reshape(4, NC, P, C).transpose(2, 0, 1, 3).reshape(P, 4 * NC * C)
    ).astype(f)
    b_pack = np.ascontiguousarray(
        bf.reshape(4, NC, P).transpose(2, 0, 1).reshape(P, 16)
    ).astype(f)

    q_in = np.ascontiguousarray(query.reshape(B, C, T).astype(f))
    k_in = np.ascontiguousarray(key.reshape(B, C, T).astype(f))
    v_in = np.ascontiguousarray(value.reshape(B, C, T).astype(f))
    in_maps = []
    for b in range(B):
        mT = np.ascontiguousarray(mask[b, 0].T.astype(np.float16))
        in_maps.append({
            "x_q": q_in[b], "x_k": k_in[b], "x_v": v_in[b],
            "m_t": mT, "w_pk": w_pack, "b_pk": b_pack,
        })
    return in_maps


def run(inputs, trace=False, **kw):
    _, _, _, _, bass_utils, _ = _get_mods()
    nc = _get_program()
    in_maps = _prep_in_maps(**inputs)
    res = bass_utils.run_bass_kernel_spmd(
        nc, in_maps, core_ids=list(range(B)), trace=trace, **kw,
    )
    out = np.stack([res.results[b]["out_d"] for b in range(B)], axis=0)
    out = out.reshape(B, C, 1, T).astype(np.float32)
    new_cache = np.concatenate(
        [np.asarray(inputs["key"], dtype=np.float32),
         np.asarray(inputs["value"], dtype=np.float32)], axis=1,
    )
    return (out, new_cache), res


def kernel(**inputs):
    outs, _ = run(inputs, trace=False)
    return outs

